# revision 5
# baseline (speedup 1.0000x reference)
"""AdaptiveMask normalize kernel for Trainium2 (8 NeuronCores, data parallel).

out = (x * mask) / (sum(x * mask, axis=-1, keepdims=True) + 1e-8)

x: (8, 8, 64, 64, 289) f32.  Sharded along batch dim: core i gets x[i]
flattened to (32768, 289).  The tiny 289-element mask is built host-side
(exact f32 replication of the reference ramp construction) and replicated
to every core.  Per core the kernel is a simple row-normalize:
fused multiply+row-sum on the vector engine (tensor_tensor_reduce),
reciprocal, then a per-partition-scale Copy activation on the scalar
engine, with big contiguous DMAs (R rows per partition per tile).
"""

import sys

import numpy as np

if "/opt/trn_rl_repo" not in sys.path:
    sys.path.insert(0, "/opt/trn_rl_repo")

P = 128                     # SBUF partitions
K2 = 289                    # (2*mask_len+1)^2
ROWS_PER_CORE = 8 * 64 * 64  # 32768 rows per batch-shard
R = 16                      # rows per partition per tile
T = ROWS_PER_CORE // (P * R)  # tiles per core
N_CORES = 8
EPS = 1e-8
RAMP_SIZE = np.float32(8.0)

_compiled_nc = None
LAST_RESULT = None  # BassKernelResults of the most recent run (for profiling)


def _build_mask_host(current_val, mask_template, mask_len):
    """Exact f32 replication of reference._build_mask, flattened to (K*K,)."""
    cv = np.float32(np.asarray(current_val).reshape(-1)[0])
    mt = np.asarray(mask_template).astype(np.float32)
    max_size = np.float32(mt.shape[0])
    one_d = (mt + cv * max_size) / RAMP_SIZE + np.float32(1.0)
    one_d = np.clip(one_d, np.float32(0.0), np.float32(1.0))[-mask_len:]
    L = mask_len
    K = 2 * L + 1
    r = np.arange(K)
    d = np.maximum(np.abs(r[:, None] - L), np.abs(r[None, :] - L))
    idx = np.clip(L - d, 0, L - 1)
    mask2d = np.where(d == 0, np.float32(1.0), one_d[idx]).astype(np.float32)
    return mask2d.reshape(K * K)


def _build_graph():
    import concourse.bacc as bacc
    import concourse.tile as tile
    from concourse import mybir

    nc = bacc.Bacc(name="adaptive_mask_norm")
    x_d = nc.dram_tensor("x", [ROWS_PER_CORE, K2], mybir.dt.float32,
                         kind="ExternalInput")
    m_d = nc.dram_tensor("mask", [1, K2], mybir.dt.float32,
                         kind="ExternalInput")
    o_d = nc.dram_tensor("out", [ROWS_PER_CORE, K2], mybir.dt.float32,
                         kind="ExternalOutput")

    x_v = x_d[:, :].rearrange("(t p r) d -> t p r d", p=P, r=R)
    o_v = o_d[:, :].rearrange("(t p r) d -> t p r d", p=P, r=R)

    with tile.TileContext(nc) as tc:
        with tc.tile_pool(name="xs", bufs=3) as xs, \
             tc.tile_pool(name="ms", bufs=3) as ms, \
             tc.tile_pool(name="st", bufs=3) as st, \
             tc.tile_pool(name="const", bufs=1) as const:
            # mask replicated to [P, R, K2] in SBUF once (tiny one-time DMA)
            mask_sb = const.tile([P, R, K2], mybir.dt.float32)
            nc.gpsimd.dma_start(
                out=mask_sb,
                in_=m_d[:, :].unsqueeze(1).to_broadcast([P, R, K2]),
            )
            for t in range(T):
                x_t = xs.tile([P, R, K2], mybir.dt.float32)
                nc.sync.dma_start(out=x_t, in_=x_v[t])
                m_t = ms.tile([P, R, K2], mybir.dt.float32)
                sums = st.tile([P, R], mybir.dt.float32)
                nc.vector.tensor_mul(m_t, x_t, mask_sb)
                nc.vector.tensor_reduce(
                    out=sums, in_=m_t,
                    axis=mybir.AxisListType.X, op=mybir.AluOpType.add)
                nc.vector.tensor_scalar_add(out=sums, in0=sums, scalar1=EPS)
                nc.vector.reciprocal(out=sums, in_=sums)
                for j in range(R):
                    nc.scalar.activation(
                        out=m_t[:, j, :],
                        in_=m_t[:, j, :],
                        func=mybir.ActivationFunctionType.Copy,
                        scale=sums[:, j:j + 1],
                    )
                nc.sync.dma_start(out=o_v[t], in_=m_t)
    nc.finalize()
    return nc


def kernel(x, current_val, mask_template, mask_len):
    global _compiled_nc, LAST_RESULT
    from concourse.bass_utils import run_bass_kernel_spmd

    x = np.asarray(x, dtype=np.float32)
    mask_len = int(np.asarray(mask_len))
    mask = _build_mask_host(current_val, mask_template, mask_len)

    if _compiled_nc is None:
        _compiled_nc = _build_graph()

    mask_2d = np.ascontiguousarray(mask.reshape(1, K2))
    in_maps = [
        {
            "x": np.ascontiguousarray(x[i]).reshape(ROWS_PER_CORE, K2),
            "mask": mask_2d,
        }
        for i in range(N_CORES)
    ]
    res = run_bass_kernel_spmd(_compiled_nc, in_maps, core_ids=list(range(N_CORES)))
    LAST_RESULT = res
    out = np.stack(
        [res.results[i]["out"].reshape(x.shape[1:]) for i in range(N_CORES)],
        axis=0,
    )
    return out.astype(np.float32, copy=False)


# ---------------------------------------------------------------------------
# Test-only helpers below (never used by the grading harness).
# ---------------------------------------------------------------------------

def _make_sharded_callable(nc, n_cores):
    """Rebuild the same shard_map'd callable run_bass_via_pjrt uses, but
    return it so repeated on-device executions can be timed."""
    import jax
    from jax.experimental.shard_map import shard_map
    from jax.sharding import Mesh, PartitionSpec

    from concourse import mybir
    from concourse.bass2jax import (
        _bass_exec_p,
        install_neuronx_cc_hook,
        partition_id_tensor,
    )

    install_neuronx_cc_hook()

    in_names, out_names, out_avals = [], [], []
    partition_name = nc.partition_id_tensor.name if nc.partition_id_tensor else None
    for alloc in nc.m.functions[0].allocations:
        if not isinstance(alloc, mybir.MemoryLocationSet):
            continue
        name = alloc.memorylocations[0].name
        if alloc.kind == "ExternalInput":
            if name != partition_name:
                in_names.append(name)
        elif alloc.kind == "ExternalOutput":
            out_names.append(name)
            out_avals.append(
                jax.core.ShapedArray(tuple(alloc.tensor_shape),
                                     mybir.dt.np(alloc.dtype))
            )
    n_params = len(in_names)
    n_outs = len(out_avals)
    in_names_full = in_names + out_names
    if partition_name is not None:
        in_names_full.append(partition_name)
    donate = tuple(range(n_params, n_params + n_outs))

    def _body(*args):
        operands = list(args)
        if partition_name is not None:
            operands.append(partition_id_tensor())
        outs = _bass_exec_p.bind(
            *operands,
            out_avals=tuple(out_avals),
            in_names=tuple(in_names_full),
            out_names=tuple(out_names),
            lowering_input_output_aliases=(),
            sim_require_finite=True,
            sim_require_nnan=True,
            nc=nc,
        )
        return tuple(outs)

    devices = jax.devices()[:n_cores]
    mesh = Mesh(np.asarray(devices), ("core",))
    in_specs = (PartitionSpec("core"),) * (n_params + n_outs)
    out_specs = (PartitionSpec("core"),) * len(out_names)
    sharded = jax.jit(
        shard_map(_body, mesh=mesh, in_specs=in_specs, out_specs=out_specs,
                  check_rep=False),
        donate_argnums=donate,
        keep_unused=True,
    )
    return sharded, mesh, in_names, out_avals


def bench_steady_state(np_inputs, iters=10):
    """Time repeated on-device executions; returns estimated exec ns/call.

    Inputs stay device-resident; donated output buffers are regenerated
    on-device outside the timed region. Reports min over iters.
    """
    import time

    import jax
    import jax.numpy as jnp
    from jax.sharding import NamedSharding, PartitionSpec

    global _compiled_nc
    if _compiled_nc is None:
        _compiled_nc = _build_graph()
    nc = _compiled_nc

    x = np.asarray(np_inputs["x"], dtype=np.float32)
    mask = _build_mask_host(
        np_inputs["current_val"], np_inputs["mask_template"],
        int(np.asarray(np_inputs["mask_len"])),
    )

    sharded, mesh, in_names, out_avals = _make_sharded_callable(nc, N_CORES)
    spec = NamedSharding(mesh, PartitionSpec("core"))

    concat = {
        "x": x.reshape(N_CORES * ROWS_PER_CORE, K2),
        "mask": np.concatenate([mask.reshape(1, K2)] * N_CORES, axis=0),
    }
    in_dev = [jax.device_put(concat[n], spec) for n in in_names]

    zero_shapes = [(N_CORES * a.shape[0], *a.shape[1:]) for a in out_avals]
    make_zeros = jax.jit(
        lambda: tuple(jnp.zeros(s, a.dtype) for s, a in zip(zero_shapes, out_avals)),
        out_shardings=tuple(spec for _ in out_avals),
    )

    times = []
    for _ in range(iters):
        zeros = make_zeros()
        jax.block_until_ready(zeros)
        t0 = time.perf_counter()
        out = sharded(*in_dev, *zeros)
        jax.block_until_ready(out)
        t1 = time.perf_counter()
        times.append(t1 - t0)
        del out
    times_ns = sorted(t * 1e9 for t in times)
    print(f"  bench times (us): {[round(t / 1e3, 1) for t in times_ns]}")
    return times_ns[0]


# revision 6
# speedup vs baseline: 2.1769x; 2.1769x over previous
"""AdaptiveMask normalize kernel for Trainium2 (8 NeuronCores, data parallel).

out = (x * mask) / (sum(x * mask, axis=-1, keepdims=True) + 1e-8)

x: (8, 8, 64, 64, 289) f32.  Sharded along batch dim: core i gets x[i]
flattened to (32768, 289).  The tiny 289-element mask is built host-side
(exact f32 replication of the reference ramp construction) and, when it is
identically 1.0 (true for the reference init current_val=0.5), the
multiply is skipped entirely — bitwise identical since x*1.0 == x.

Per core: tiles of 128 partitions x R rows x 289, big contiguous DMAs,
row-sum on the vector engine (tensor_reduce, matches the jax-on-neuron
reference bit-exactly), reciprocal, then per-partition-scale Copy
activations on the scalar engine, all in place in one deep-buffered pool.
"""

import sys

import numpy as np

if "/opt/trn_rl_repo" not in sys.path:
    sys.path.insert(0, "/opt/trn_rl_repo")

P = 128                      # SBUF partitions
K2 = 289                     # (2*mask_len+1)^2
ROWS_PER_CORE = 8 * 64 * 64  # 32768 rows per batch-shard
R = 16                       # rows per partition per tile
T = ROWS_PER_CORE // (P * R)  # tiles per core
N_CORES = 8
EPS = 1e-8
RAMP_SIZE = np.float32(8.0)
BUFS = 6

_compiled = {}
LAST_RESULT = None


def _build_mask_host(current_val, mask_template, mask_len):
    """Exact f32 replication of reference._build_mask, flattened to (K*K,)."""
    cv = np.float32(np.asarray(current_val).reshape(-1)[0])
    mt = np.asarray(mask_template).astype(np.float32)
    max_size = np.float32(mt.shape[0])
    one_d = (mt + cv * max_size) / RAMP_SIZE + np.float32(1.0)
    one_d = np.clip(one_d, np.float32(0.0), np.float32(1.0))[-mask_len:]
    L = mask_len
    K = 2 * L + 1
    r = np.arange(K)
    d = np.maximum(np.abs(r[:, None] - L), np.abs(r[None, :] - L))
    idx = np.clip(L - d, 0, L - 1)
    mask2d = np.where(d == 0, np.float32(1.0), one_d[idx]).astype(np.float32)
    return mask2d.reshape(K * K)


def _build_graph(apply_mask, repeat=0):
    """Build the per-core SPMD graph.

    apply_mask: multiply by the mask tensor (False when mask == 1.0).
    repeat: 0 for the normal graph; >0 wraps the whole sweep in a For_i
        for on-device timing calibration (test-only).
    """
    import concourse.bacc as bacc
    import concourse.tile as tile
    from concourse import mybir

    nc = bacc.Bacc(name=f"adamask_m{int(apply_mask)}_r{repeat}")
    x_d = nc.dram_tensor("x", [ROWS_PER_CORE, K2], mybir.dt.float32,
                         kind="ExternalInput")
    if apply_mask:
        m_d = nc.dram_tensor("mask", [1, K2], mybir.dt.float32,
                             kind="ExternalInput")
    o_d = nc.dram_tensor("out", [ROWS_PER_CORE, K2], mybir.dt.float32,
                         kind="ExternalOutput")

    x_v = x_d[:, :].rearrange("(t p r) d -> t p r d", p=P, r=R)
    o_v = o_d[:, :].rearrange("(t p r) d -> t p r d", p=P, r=R)

    with tile.TileContext(nc) as tc:
        with tc.tile_pool(name="xs", bufs=BUFS) as xs, \
             tc.tile_pool(name="st", bufs=BUFS) as st, \
             tc.tile_pool(name="const", bufs=1) as const:
            if apply_mask:
                mask_sb = const.tile([P, R, K2], mybir.dt.float32)
                nc.gpsimd.dma_start(
                    out=mask_sb,
                    in_=m_d[:, :].unsqueeze(1).to_broadcast([P, R, K2]),
                )

            def body(_iv=None):
                for t in range(T):
                    x_t = xs.tile([P, R, K2], mybir.dt.float32)
                    nc.sync.dma_start(out=x_t, in_=x_v[t])
                    sums = st.tile([P, R], mybir.dt.float32)
                    if apply_mask:
                        nc.vector.tensor_mul(x_t, x_t, mask_sb)
                    nc.vector.tensor_reduce(
                        out=sums, in_=x_t,
                        axis=mybir.AxisListType.X, op=mybir.AluOpType.add)
                    nc.vector.tensor_scalar_add(out=sums, in0=sums, scalar1=EPS)
                    nc.vector.reciprocal(out=sums, in_=sums)
                    for j in range(R):
                        nc.scalar.activation(
                            out=x_t[:, j, :],
                            in_=x_t[:, j, :],
                            func=mybir.ActivationFunctionType.Copy,
                            scale=sums[:, j:j + 1],
                        )
                    nc.sync.dma_start(out=o_v[t], in_=x_t)

            if repeat:
                with tc.For_i(0, repeat, 1) as _i:
                    body(_i)
            else:
                body()
    nc.finalize()
    return nc


def _get_graph(apply_mask, repeat=0):
    key = (bool(apply_mask), int(repeat))
    if key not in _compiled:
        _compiled[key] = _build_graph(apply_mask, repeat)
    return _compiled[key]


def kernel(x, current_val, mask_template, mask_len):
    global LAST_RESULT
    from concourse.bass_utils import run_bass_kernel_spmd

    x = np.asarray(x, dtype=np.float32)
    mask_len = int(np.asarray(mask_len))
    mask = _build_mask_host(current_val, mask_template, mask_len)
    apply_mask = not bool(np.all(mask == np.float32(1.0)))

    nc = _get_graph(apply_mask)

    in_maps = []
    mask_2d = np.ascontiguousarray(mask.reshape(1, K2))
    for i in range(N_CORES):
        m = {"x": np.ascontiguousarray(x[i]).reshape(ROWS_PER_CORE, K2)}
        if apply_mask:
            m["mask"] = mask_2d
        in_maps.append(m)
    res = run_bass_kernel_spmd(nc, in_maps, core_ids=list(range(N_CORES)))
    LAST_RESULT = res
    out = np.stack(
        [res.results[i]["out"].reshape(x.shape[1:]) for i in range(N_CORES)],
        axis=0,
    )
    return out.astype(np.float32, copy=False)


# ---------------------------------------------------------------------------
# Test-only helpers below (never used by the grading harness).
# ---------------------------------------------------------------------------

def _run_once(nc, np_inputs, apply_mask):
    from concourse.bass_utils import run_bass_kernel_spmd

    x = np.asarray(np_inputs["x"], dtype=np.float32)
    mask = _build_mask_host(
        np_inputs["current_val"], np_inputs["mask_template"],
        int(np.asarray(np_inputs["mask_len"])))
    in_maps = []
    for i in range(N_CORES):
        m = {"x": np.ascontiguousarray(x[i]).reshape(ROWS_PER_CORE, K2)}
        if apply_mask:
            m["mask"] = np.ascontiguousarray(mask.reshape(1, K2))
        in_maps.append(m)
    return run_bass_kernel_spmd(nc, in_maps, core_ids=list(range(N_CORES)))


def bench_repeat(np_inputs, k_lo=1, k_hi=33, runs=3):
    """On-device repeat-loop timing: exec_ns = (wall(k_hi) - wall(k_lo)) /
    (k_hi - k_lo), min over `runs`. Removes all dispatch overhead."""
    import time

    mask = _build_mask_host(
        np_inputs["current_val"], np_inputs["mask_template"],
        int(np.asarray(np_inputs["mask_len"])))
    apply_mask = not bool(np.all(mask == np.float32(1.0)))

    nc_lo = _get_graph(apply_mask, repeat=k_lo)
    nc_hi = _get_graph(apply_mask, repeat=k_hi)

    def time_graph(nc):
        best = None
        for _ in range(runs):
            t0 = time.perf_counter()
            _run_once(nc, np_inputs, apply_mask)
            t1 = time.perf_counter()
            dt = t1 - t0
            best = dt if best is None else min(best, dt)
        return best

    # warm both (compile/caches)
    time_graph(nc_lo)
    time_graph(nc_hi)
    w_lo = time_graph(nc_lo)
    w_hi = time_graph(nc_hi)
    exec_ns = (w_hi - w_lo) * 1e9 / (k_hi - k_lo)
    print(f"  wall lo(k={k_lo}): {w_lo * 1e3:.1f} ms   "
          f"hi(k={k_hi}): {w_hi * 1e3:.1f} ms")
    return exec_ns


# revision 12
# speedup vs baseline: 238.0643x; 109.3571x over previous
"""AdaptiveMask normalize kernel for Trainium2 (8 NeuronCores, data parallel).

out = (x * mask) / (sum(x * mask, axis=-1, keepdims=True) + 1e-8)

x: (8, 8, 64, 64, 289) f32.  Sharded along batch dim: core i gets x[i]
flattened to (32768, 289).  The tiny 289-element mask is built host-side
(exact f32 replication of the reference ramp construction) and, when it is
identically 1.0 (true for the reference init current_val=0.5), the
multiply is skipped entirely — bitwise identical since x*1.0 == x.

Per core: tiles of 128 partitions x R rows x 289, big contiguous DMAs,
row-sum on the vector engine (tensor_reduce, matches the jax-on-neuron
reference bit-exactly), reciprocal, then per-partition-scale Copy
activations on the scalar engine, all in place in one deep-buffered pool.
"""

import sys

import numpy as np

if "/opt/trn_rl_repo" not in sys.path:
    sys.path.insert(0, "/opt/trn_rl_repo")

P = 128                      # SBUF partitions
K2 = 289                     # (2*mask_len+1)^2
ROWS_PER_CORE = 8 * 64 * 64  # 32768 rows per batch-shard
R = 32                       # rows per partition per tile
T = ROWS_PER_CORE // (P * R)  # tiles per core
N_CORES = 8
EPS = 1e-8
RAMP_SIZE = np.float32(8.0)
BUFS = 4
OUT_ENG = "scalar"           # store DMAs on the ACT HWDGE FIFO (loads on SP)

_compiled = {}
LAST_RESULT = None


def _build_mask_host(current_val, mask_template, mask_len):
    """Exact f32 replication of reference._build_mask, flattened to (K*K,)."""
    cv = np.float32(np.asarray(current_val).reshape(-1)[0])
    mt = np.asarray(mask_template).astype(np.float32)
    max_size = np.float32(mt.shape[0])
    one_d = (mt + cv * max_size) / RAMP_SIZE + np.float32(1.0)
    one_d = np.clip(one_d, np.float32(0.0), np.float32(1.0))[-mask_len:]
    L = mask_len
    K = 2 * L + 1
    r = np.arange(K)
    d = np.maximum(np.abs(r[:, None] - L), np.abs(r[None, :] - L))
    idx = np.clip(L - d, 0, L - 1)
    mask2d = np.where(d == 0, np.float32(1.0), one_d[idx]).astype(np.float32)
    return mask2d.reshape(K * K)


def _build_graph(apply_mask, repeat=0, r=R, bufs=BUFS, out_eng=OUT_ENG):
    """Build the per-core SPMD graph.

    apply_mask: multiply by the mask tensor (False when mask == 1.0).
    repeat: 0 for the normal graph; >0 wraps the whole sweep in a For_i
        for on-device timing calibration (test-only).
    out_eng: engine issuing the store DMAs ("sync" shares the SP HWDGE
        FIFO with loads; "scalar" uses the ACT HWDGE FIFO; "gpsimd" SWDGE).
    """
    import concourse.bacc as bacc
    import concourse.tile as tile
    from concourse import mybir

    t_count = ROWS_PER_CORE // (P * r)
    nc = bacc.Bacc(
        name=f"adamask_m{int(apply_mask)}_r{repeat}_R{r}_b{bufs}_{out_eng}")
    x_d = nc.dram_tensor("x", [ROWS_PER_CORE, K2], mybir.dt.float32,
                         kind="ExternalInput")
    if apply_mask:
        m_d = nc.dram_tensor("mask", [1, K2], mybir.dt.float32,
                             kind="ExternalInput")
    o_d = nc.dram_tensor("out", [ROWS_PER_CORE, K2], mybir.dt.float32,
                         kind="ExternalOutput")

    x_v = x_d[:, :].rearrange("(t p r) d -> t p r d", p=P, r=r)
    o_v = o_d[:, :].rearrange("(t p r) d -> t p r d", p=P, r=r)

    out_engine = {"sync": nc.sync, "scalar": nc.scalar,
                  "gpsimd": nc.gpsimd}[out_eng]
    with tile.TileContext(nc) as tc:
        with tc.tile_pool(name="xs", bufs=bufs) as xs, \
             tc.tile_pool(name="st", bufs=bufs) as st, \
             tc.tile_pool(name="const", bufs=1) as const:
            if apply_mask:
                mask_sb = const.tile([P, r, K2], mybir.dt.float32)
                nc.gpsimd.dma_start(
                    out=mask_sb,
                    in_=m_d[:, :].unsqueeze(1).to_broadcast([P, r, K2]),
                )

            def body(_iv=None):
                for t in range(t_count):
                    x_t = xs.tile([P, r, K2], mybir.dt.float32)
                    nc.sync.dma_start(out=x_t, in_=x_v[t])
                    sums = st.tile([P, r], mybir.dt.float32)
                    if apply_mask:
                        nc.vector.tensor_mul(x_t, x_t, mask_sb)
                    nc.vector.tensor_reduce(
                        out=sums, in_=x_t,
                        axis=mybir.AxisListType.X, op=mybir.AluOpType.add)
                    nc.vector.tensor_scalar_add(out=sums, in0=sums, scalar1=EPS)
                    nc.vector.reciprocal(out=sums, in_=sums)
                    for j in range(r):
                        nc.scalar.activation(
                            out=x_t[:, j, :],
                            in_=x_t[:, j, :],
                            func=mybir.ActivationFunctionType.Copy,
                            scale=sums[:, j:j + 1],
                        )
                    out_engine.dma_start(out=o_v[t], in_=x_t)

            if repeat:
                with tc.For_i(0, repeat, 1) as _i:
                    body(_i)
            else:
                body()
    nc.finalize()
    return nc


def _get_graph(apply_mask, repeat=0, r=R, bufs=BUFS, out_eng=OUT_ENG):
    key = (bool(apply_mask), int(repeat), int(r), int(bufs), out_eng)
    if key not in _compiled:
        _compiled[key] = _build_graph(apply_mask, repeat, r, bufs, out_eng)
    return _compiled[key]


def kernel(x, current_val, mask_template, mask_len):
    global LAST_RESULT
    from concourse.bass_utils import run_bass_kernel_spmd

    x = np.asarray(x, dtype=np.float32)
    mask_len = int(np.asarray(mask_len))
    mask = _build_mask_host(current_val, mask_template, mask_len)
    apply_mask = not bool(np.all(mask == np.float32(1.0)))

    nc = _get_graph(apply_mask)

    in_maps = []
    mask_2d = np.ascontiguousarray(mask.reshape(1, K2))
    for i in range(N_CORES):
        m = {"x": np.ascontiguousarray(x[i]).reshape(ROWS_PER_CORE, K2)}
        if apply_mask:
            m["mask"] = mask_2d
        in_maps.append(m)
    res = run_bass_kernel_spmd(nc, in_maps, core_ids=list(range(N_CORES)))
    LAST_RESULT = res
    out = np.stack(
        [res.results[i]["out"].reshape(x.shape[1:]) for i in range(N_CORES)],
        axis=0,
    )
    return out.astype(np.float32, copy=False)


# ---------------------------------------------------------------------------
# Test-only helpers below (never used by the grading harness).
# ---------------------------------------------------------------------------

def _run_once(nc, np_inputs, apply_mask):
    from concourse.bass_utils import run_bass_kernel_spmd

    x = np.asarray(np_inputs["x"], dtype=np.float32)
    mask = _build_mask_host(
        np_inputs["current_val"], np_inputs["mask_template"],
        int(np.asarray(np_inputs["mask_len"])))
    in_maps = []
    for i in range(N_CORES):
        m = {"x": np.ascontiguousarray(x[i]).reshape(ROWS_PER_CORE, K2)}
        if apply_mask:
            m["mask"] = np.ascontiguousarray(mask.reshape(1, K2))
        in_maps.append(m)
    return run_bass_kernel_spmd(nc, in_maps, core_ids=list(range(N_CORES)))


def bench_repeat(np_inputs, k_lo=1, k_hi=16385, runs=5):
    """On-device repeat-loop timing: exec_ns per sweep from the slope of
    interleaved k_lo/k_hi runs (medians). Removes dispatch overhead."""
    import statistics
    import time

    mask = _build_mask_host(
        np_inputs["current_val"], np_inputs["mask_template"],
        int(np.asarray(np_inputs["mask_len"])))
    apply_mask = not bool(np.all(mask == np.float32(1.0)))

    nc_lo = _get_graph(apply_mask, repeat=k_lo)
    nc_hi = _get_graph(apply_mask, repeat=k_hi)

    # warm both (compile/caches)
    _run_once(nc_lo, np_inputs, apply_mask)
    _run_once(nc_hi, np_inputs, apply_mask)
    lo_t, hi_t = [], []
    for _ in range(runs):
        for nc, acc in ((nc_lo, lo_t), (nc_hi, hi_t)):
            t0 = time.perf_counter()
            _run_once(nc, np_inputs, apply_mask)
            acc.append(time.perf_counter() - t0)
    w_lo = statistics.median(lo_t)
    w_hi = statistics.median(hi_t)
    exec_ns = (w_hi - w_lo) * 1e9 / (k_hi - k_lo)
    print(f"  wall lo(k={k_lo}): {w_lo * 1e3:.1f} ms   "
          f"hi(k={k_hi}): {w_hi * 1e3:.1f} ms")
    return exec_ns
